# revision 3
# baseline (speedup 1.0000x reference)
"""FlowNet-style correlation layer (B=4, C=128, H=W=192, k=9, stride=1) on 8 trn2 cores.

Design (per core; cores = 4 batches x 2 H-halves, SPMD):
  - Host pre-blocks x into per-patch-contiguous layout [c, blk, 128] (bf16,
    pre-scaled by 1/C — exact in bf16) and pads y to [c, 104, 200] (bf16,
    h-major, zero-padded W and halo rows from the neighboring half).
  - Device: per 8x16 pixel patch (144 blocks), one PE matmul contracting
    channels: lhsT = x-patch [c, 128], rhs = strided 3D view of resident y
    [c, 16h, 24w] -> psum[128, 384] ("banded all-pairs": psum[m, n] =
    sum_c x[c,pix_m] * y[c,ctx_n]).
  - Evacuate psum -> staging SBUF bf16 (alternating ACT/DVE); per G=16
    blocks, flush with 8 per-hl partition-range DMAs: pixel rows
    [16hl,16hl+16) keep only cols [24hl, 24hl+216) of each 384 band
    (the useful window is uniform within an hl group), out 14.2->8MB.
  - Inputs are loaded in row-chunks so compute overlaps the load.
  - Host extracts the 81 useful offsets per pixel from the 384-wide band
    (numpy gather; n = (hl+i)*24 + (wl+j) for pixel (hl,wl), offset (i,j))
    and reassembles [B, 81, 192, 192] f32.

This keeps total DMA instructions ~O(50) (the v1 sheared-scratch design
dispatched ~1300 DMAs at ~600ns each on the sync engine = 786us serial).
"""

import numpy as np

B, C, H, W = 4, 128, 192, 192
K = 9                      # kernel_size
PAD = 4                    # displacement radius
NCORES = 8
HSH = H // 2               # 96 rows per core
YH, YW = HSH + 2 * PAD, W + 2 * PAD       # 104, 200
PH, PW = 8, 16             # patch shape (128 pixels)
CH, CW = PH + 2 * PAD, PW + 2 * PAD       # context 16 x 24
NCTX = CH * CW             # 384 band columns
NQ = (PW - 1) + (K - 1) * CW + K          # 216 useful cols per pixel
NBH, NBW = HSH // PH, W // PW             # 12 x 12 = 144 blocks
NBLK = NBH * NBW
K2 = K * K                 # 81
FLUSH = 16                 # blocks per output flush
NGRP = NBLK // FLUSH       # 9 output groups

_nc_cache = None


def _build_nc():
    import concourse.bacc as bacc
    import concourse.mybir as mybir
    import concourse.tile as tile

    bf16 = mybir.dt.bfloat16
    f32 = mybir.dt.float32

    nc = bacc.Bacc("TRN2", target_bir_lowering=False, debug=False)
    x_d = nc.dram_tensor("x", [C, NBLK * 128], bf16, kind="ExternalInput")
    y_d = nc.dram_tensor("y", [C, YH * YW], bf16, kind="ExternalInput")
    out_d = nc.dram_tensor("out", [NGRP, PH * PW * FLUSH * NQ], bf16,
                           kind="ExternalOutput")

    with tile.TileContext(nc) as tc:
        with (
            tc.tile_pool(name="resident", bufs=1) as res_pool,
            tc.tile_pool(name="psum", bufs=8, space="PSUM") as psum_pool,
            tc.tile_pool(name="stage", bufs=2) as stage_pool,
        ):
            x_sb = res_pool.tile([C, NBLK * 128], bf16)
            y_sb = res_pool.tile([C, YH * YW], bf16)

            # chunked loads (8 y rows / one x block-row per DMA) so the
            # first block rows can start while the tail still streams in
            YCH = 8 * YW                       # 1600 cols per y chunk
            XCH = NBW * 128                    # 1536 cols per x chunk
            for i in range(2):
                nc.sync.dma_start(y_sb[:, i * YCH:(i + 1) * YCH],
                                  y_d[:, i * YCH:(i + 1) * YCH])
            for bh in range(NBH):
                nc.sync.dma_start(x_sb[:, bh * XCH:(bh + 1) * XCH],
                                  x_d[:, bh * XCH:(bh + 1) * XCH])
                if bh + 2 < YH // 8:
                    i = bh + 2
                    nc.sync.dma_start(y_sb[:, i * YCH:(i + 1) * YCH],
                                      y_d[:, i * YCH:(i + 1) * YCH])

            y3 = y_sb[:].rearrange("c (h w) -> c h w", w=YW)

            blk = 0
            for bh in range(NBH):
                for bw in range(NBW):
                    j = blk % FLUSH
                    if j == 0:
                        stage = stage_pool.tile([128, FLUSH * NCTX], bf16)

                    lhsT = x_sb[:, blk * 128:(blk + 1) * 128]
                    rhs = y3[:, PH * bh:PH * bh + CH, PW * bw:PW * bw + CW]
                    ps = psum_pool.tile([128, NCTX], f32)
                    nc.tensor.matmul(ps[:], lhsT, rhs, start=True, stop=True)

                    dst = stage[:, j * NCTX:(j + 1) * NCTX]
                    if blk % 2 == 0:
                        nc.scalar.activation(
                            dst, ps[:], mybir.ActivationFunctionType.Copy)
                    else:
                        nc.vector.tensor_copy(dst, ps[:])

                    if j == FLUSH - 1:
                        grp = blk // FLUSH
                        HSZ = PW * FLUSH * NQ      # elems per hl chunk
                        for hl in range(PH):
                            ssrc = stage[PW * hl:PW * (hl + 1), :]
                            ssrc = ssrc.rearrange(
                                "p (g n) -> p g n", g=FLUSH)[
                                :, :, CW * hl:CW * hl + NQ]
                            eng = nc.sync if hl % 2 == 0 else nc.scalar
                            eng.dma_start(
                                out_d[grp:grp + 1,
                                      hl * HSZ:(hl + 1) * HSZ],
                                ssrc)
                    blk += 1

    nc.compile()
    return nc


def _get_nc():
    global _nc_cache
    if _nc_cache is None:
        _nc_cache = _build_nc()
    return _nc_cache


def shard_inputs(x, y):
    import ml_dtypes
    xs_all = (np.asarray(x) * np.float32(1.0 / C)).astype(ml_dtypes.bfloat16)
    yp = np.pad(np.asarray(y).astype(ml_dtypes.bfloat16),
                ((0, 0), (0, 0), (PAD, PAD), (PAD, PAD)))
    in_maps = []
    for b in range(B):
        for hh in range(2):
            xs = xs_all[b, :, hh * HSH:(hh + 1) * HSH, :]     # [c, 96, 192]
            # pre-block: [c, bh, hl, bw, wl] -> [c, (bh bw), (hl wl)]
            xs = xs.reshape(C, NBH, PH, NBW, PW).transpose(0, 1, 3, 2, 4)
            xs = np.ascontiguousarray(xs.reshape(C, NBLK * 128))
            ys = yp[b, :, hh * HSH:hh * HSH + YH, :]          # [c, 104, 200]
            ys = np.ascontiguousarray(ys.reshape(C, YH * YW))
            in_maps.append({"x": xs, "y": ys})
    return in_maps


# per-pixel offset into the 216-col window (24*hl removed per hl
# group): pixel m = hl*PW + wl reads cols wl + i*CW + j, k = i*K + j
_WL = np.tile(np.arange(PW), PH)          # [128]
_POS = _WL.astype(np.int64)               # [128]
_OFF = (np.arange(K)[:, None] * CW + np.arange(K)).ravel()  # [81]
_IDX = np.broadcast_to((_POS[:, None] + _OFF[None, :])[:, None, :],
                       (128, NBLK, K2))


def unshard_output(results):
    out = np.empty((B, K2, H, W), np.float32)
    for core, r in enumerate(results):
        band = np.asarray(r["out"]).reshape(NGRP, PH, PW, FLUSH, NQ)
        band = band.transpose(1, 2, 0, 3, 4).reshape(128, NBLK, NQ)
        o = np.take_along_axis(band, _IDX, axis=2).astype(np.float32)
        b, hh = divmod(core, 2)
        o = o.reshape(PH, PW, NBH, NBW, K2)      # [hl, wl, bh, bw, k]
        o = o.transpose(4, 2, 0, 3, 1).reshape(K2, HSH, W)
        out[b, :, hh * HSH:(hh + 1) * HSH, :] = o
    return out


def kernel(x, y, kernel_size, stride, _trace=False):
    assert int(kernel_size) == K and int(stride) == 1
    from concourse.bass_utils import run_bass_kernel_spmd
    nc = _get_nc()
    in_maps = shard_inputs(x, y)
    try:
        res = run_bass_kernel_spmd(nc, in_maps, list(range(NCORES)),
                                   trace=_trace)
    except Exception:
        if not _trace:
            raise
        res = run_bass_kernel_spmd(nc, in_maps, list(range(NCORES)))
    out = unshard_output(res.results)
    if _trace:
        return out, res
    return out
